# revision 39
# baseline (speedup 1.0000x reference)
"""Trainium2 Bass kernel for nn_CompressedSensingInception.

Strategy (pure data parallel over batch, 8 NeuronCores):
- FISTA (100 iters, the dominant cost): each core owns 8 samples x 3 channels
  = 24 sparse-code columns. State y lives in SBUF as [128 part (s within
  chunk), 41*24 free (chunk, pair)], s padded 5184->5248.
    mm1  proj = mat^T y : per chunk, stationary = y-chunk [128,24],
         streamed = mat-chunk [128,81], accumulated in PSUM [24,81].
    mm2  re = mat (im - proj): PE-transpose proj->[81,24], d = imT - projT,
         per chunk stationary = matT-chunk [81,128], rhs = d [81,24].
    soft-threshold + momentum fused into 8 DVE + 4 ACT ops per iter.
- Epilogue per core: 41 PE transposes build xi_padT [24(n,c), 73*73]
  (reflect-padded); bn_x stats via one 24-byte AllReduce; conv5 as 25
  block-diagonal taps accumulating in PSUM; maxpool via strided-view
  tensor_reduce; 1x1 conv block-diag.
- w/y/z paths are tiny and need full-batch BN stats, so every core computes
  them redundantly for the whole batch from the full x (shipped replicated;
  xP built on-device via 3 PE transposes of the [64,243] x tile).
- z path: down-conv1 = 3 PSUM-accumulated matmuls against a block-sparse
  [81,324] weight; conv2/up-convs are plain matmuls with host-reordered
  weights, so no zim/concat shuffle DMAs are needed.

Dispatch (dominates wall time through the axon tunnel, ~42ms RTT):
- persistent jax.jit(shard_map(bass_exec)) built once per process; all
  weight-derived constants are device-resident across calls.
- per call ONE input ships: xc [72,243]/core = full x (replicated) + the
  core's imT slice; output operand buffers persist (no donation — the
  kernel fully writes every region the host reads).
- everything lands in ONE packed output (xi is AllGathered on-device into
  core 0's [128,273] tensor) and only core 0's shard is fetched, so a call
  costs a single synchronous round trip (~57-62ms total; device exec ~3ms,
  of which FISTA ~2ms).
- an x AllGather (shipping x sharded) was tried and is ~30ms SLOWER: the
  early cross-core rendezvous + collective overhead dwarf the 500KB saved.
"""
import os
import numpy as np
from contextlib import ExitStack

import concourse.bass as bass
import concourse.tile as tile
from concourse import bacc, mybir

F32 = np.float32
DT = mybir.dt.float32
ITERS, LAM, MU = 100, 0.005, 1.0
B, NCORES = 64, 8
NSH = B // NCORES            # 8 samples/core
NPAIR = NSH * 3              # 24 pairs/core
SCH = 41                     # s-chunks of 128
SPAD = SCH * 128             # 5248
THR = float(LAM / MU)
GRP = [(0, 21), (21, 20)]    # mm2 chunk groups (start, count)

_CACHE = {}


# ---------------------------------------------------------------- host side
def _host_shared(inputs):
    c = {}
    mat = np.asarray(inputs['mat'], F32)
    matp = np.zeros((SPAD, 81), F32); matp[:5184] = mat
    c['mat_sb'] = np.ascontiguousarray(
        matp.reshape(SCH, 128, 81).transpose(1, 0, 2).reshape(128, SCH * 81))
    c['matT_sb'] = np.ascontiguousarray(matp.T)

    t = F32(1.0); coefs = []
    for _ in range(ITERS):
        t_n = F32((F32(1.0) + np.sqrt(F32(1.0) + F32(4.0) * t * t, dtype=F32)) / F32(2.0))
        coefs.append(float(F32((t - F32(1.0)) / t_n))); t = t_n
    c['coefs'] = coefs

    w5 = np.asarray(inputs['w5'], F32)
    taps = np.zeros((25, NPAIR, NSH * 8), F32)
    for dy in range(5):
        for dx in range(5):
            for n in range(NSH):
                taps[dy * 5 + dx, n * 3:n * 3 + 3, n * 8:n * 8 + 8] = w5[dy, dx]
    c['w5taps'] = np.ascontiguousarray(taps.transpose(1, 0, 2).reshape(NPAIR, 25 * NSH * 8))
    c['b5_bc'] = np.tile(np.asarray(inputs['b5'], F32), NSH).reshape(NSH * 8, 1)

    wx2 = np.asarray(inputs['wx2'], F32).reshape(8, 2)
    wx2e = np.zeros((NSH * 8, NSH * 2), F32)
    for n in range(NSH):
        wx2e[n * 8:n * 8 + 8, n * 2:n * 2 + 2] = wx2
    c['wx2e'] = wx2e
    c['bx2_bc'] = np.tile(np.asarray(inputs['bx2'], F32), NSH).reshape(NSH * 2, 1)

    C3 = np.zeros((NPAIR, 3), F32)
    for p in range(NPAIR):
        C3[p, p % 3] = 1.0
    c['C3sel'] = C3
    c['C3selT'] = np.ascontiguousarray(C3.T)

    wy7 = np.asarray(inputs['wy7'], F32)[:, :, 0, 0]
    K7 = np.zeros((81, 81), F32)
    for yi in range(9):
        for xi_ in range(9):
            for yo in range(9):
                for xo in range(9):
                    dy, dx = yi - yo + 3, xi_ - xo + 3
                    if 0 <= dy < 7 and 0 <= dx < 7:
                        K7[yi * 9 + xi_, yo * 9 + xo] = wy7[dy, dx]
    c['K7'] = K7

    # z-path down-conv 1 as 3 PSUM-accumulated matmuls over xP [81,(c b)]:
    # W1big[(y x), c*108 + (j yo xo)] = wd1[y%3, x%3, c, j], yo=y//3, xo=x//3
    wd1 = np.asarray(inputs['wd1'], F32)
    W1 = np.zeros((81, 3, 108), F32)
    for y in range(9):
        for x_ in range(9):
            for ci in range(3):
                W1[y * 9 + x_, ci, np.arange(12) * 9 + (y // 3) * 3 + x_ // 3] = \
                    wd1[y % 3, x_ % 3, ci]
    c['W1big'] = np.ascontiguousarray(W1.reshape(81, 324))
    # wd2 reordered for z1f rows (j, k): row j*9+k, k=(dy2,dx2)
    wd2 = np.asarray(inputs['wd2'], F32)
    wd2r = np.zeros((108, 24), F32)
    for dy2 in range(3):
        for dx2 in range(3):
            for j in range(12):
                wd2r[j * 9 + dy2 * 3 + dx2] = wd2[dy2, dx2, j]
    c['wd2r'] = wd2r
    wu1 = np.asarray(inputs['wu1'], F32)[::-1, ::-1]
    c['wu1r'] = np.ascontiguousarray(wu1.transpose(2, 0, 1, 3).reshape(24, 108))
    SU = np.zeros((108, 12), F32)
    for p in range(108):
        SU[p, p % 12] = 1.0
    c['SU'] = SU
    c['SUT'] = np.ascontiguousarray(SU.T)
    SZ1 = np.zeros((108, 12), F32)
    SZ1[np.arange(108), np.arange(108) // 9] = 1.0
    c['SZ1'] = SZ1
    c['SZ1T'] = np.ascontiguousarray(SZ1.T)
    wu2 = np.asarray(inputs['wu2'], F32)[:, :, :, 0]
    WU2 = np.zeros((216, 81), F32)
    for po in range(81):
        yo, xo = po // 9, po % 9
        Y, dy, X, dx = yo // 3, yo % 3, xo // 3, xo % 3
        for c24 in range(24):
            WU2[(Y * 3 + X) * 24 + c24, po] = wu2[2 - dy, 2 - dx, c24]
    # split concat [z_up, z1]: zuf rows (kk,cu), z1f rows (j,kk)
    WU2up = np.zeros((108, 81), F32)
    WU2z1 = np.zeros((108, 81), F32)
    for kk in range(9):
        WU2up[kk * 12:kk * 12 + 12] = WU2[kk * 24:kk * 24 + 12]
        for j in range(12):
            WU2z1[j * 9 + kk] = WU2[kk * 24 + 12 + j]
    c['WU2up'] = WU2up
    c['WU2z1'] = WU2z1

    sw = np.zeros((81, 9), F32)
    vals = [*np.asarray(inputs['ww1'], F32).ravel(), float(np.asarray(inputs['bw1'], F32)[0]),
            *np.asarray(inputs['wy1'], F32).ravel(), float(np.asarray(inputs['by1'], F32)[0]),
            float(np.asarray(inputs['by7'], F32)[0])]
    for j, v in enumerate(vals):
        sw[:, j] = v
    c['smallw'] = sw
    c['ones81'] = np.ones((81, 1), F32)
    c['onesT81'] = np.ones((1, 81), F32)
    c['ident'] = np.eye(128, dtype=F32)
    c['bn_x_gb'] = np.stack([np.asarray(inputs['bn_x_g'], F32),
                             np.asarray(inputs['bn_x_b'], F32)], axis=1)
    c['bn_y_gb'] = np.array([[float(np.asarray(inputs['bn_y_g'], F32)[0]),
                              float(np.asarray(inputs['bn_y_b'], F32)[0])]], F32)
    c['bnd1_gb'] = np.stack([np.asarray(inputs['bnd1_g'], F32),
                             np.asarray(inputs['bnd1_b'], F32)], axis=1)
    c['bnd2_gb'] = np.stack([np.asarray(inputs['bnd2_g'], F32),
                             np.asarray(inputs['bnd2_b'], F32)], axis=1)
    c['bnu1_gb'] = np.stack([np.asarray(inputs['bnu1_g'], F32),
                             np.asarray(inputs['bnu1_b'], F32)], axis=1)
    return c


SHARED_IN = [
    ('mat_sb', (128, SCH * 81)), ('matT_sb', (81, SPAD)),
    ('w5taps', (NPAIR, 25 * NSH * 8)), ('b5_bc', (NSH * 8, 1)),
    ('wx2e', (NSH * 8, NSH * 2)), ('bx2_bc', (NSH * 2, 1)),
    ('C3sel', (NPAIR, 3)), ('C3selT', (3, NPAIR)),
    ('K7', (81, 81)),
    ('W1big', (81, 324)), ('wd2r', (108, 24)), ('wu1r', (24, 108)),
    ('SU', (108, 12)), ('SUT', (12, 108)),
    ('SZ1', (108, 12)), ('SZ1T', (12, 108)),
    ('WU2up', (108, 81)), ('WU2z1', (108, 81)),
    ('smallw', (81, 9)),
    ('ones81', (81, 1)), ('onesT81', (1, 81)), ('ident', (128, 128)),
    ('bn_x_gb', (3, 2)), ('bn_y_gb', (1, 2)),
    ('bnd1_gb', (12, 2)), ('bnd2_gb', (24, 2)), ('bnu1_gb', (12, 2)),
]


# -------------------------------------------------------------- device build
def _build(iters=ITERS, coefs=None, world=NCORES, r32=False):
    AT = mybir.ActivationFunctionType
    OP = mybir.AluOpType
    mc = (lambda ap: ap.bitcast(mybir.dt.float32r)) if r32 else (lambda ap: ap)
    nc = bacc.Bacc("TRN2", target_bir_lowering=False, debug=False,
                   num_devices=world)

    din = {}
    for name, shape in SHARED_IN:
        din[name] = nc.dram_tensor(name, list(shape), DT, kind="ExternalInput")
    # per-call input, one tensor: rows 0:64 = full x (replicated across
    # cores), rows 64:72 = this core's imT [81,24] flattened (1944=8*243)
    din['xc'] = nc.dram_tensor('xc', [B + NSH, 243], DT, kind="ExternalInput")
    # single packed output: rows 0:128 cols 0:81 = AllGathered xi (core-major),
    # cols 81:145/145:209/209:273 rows 0:81 = w/y/z [81,B]
    out_all = nc.dram_tensor('out_all', [128, 273], DT, kind="ExternalOutput")

    with tile.TileContext(nc) as tc, ExitStack() as ctx:
        consts = ctx.enter_context(tc.tile_pool(name="consts", bufs=1))
        sb = {}
        for name, shape in SHARED_IN:
            sb[name] = consts.tile(list(shape), DT, tag=name, name=f"c_{name}")
            nc.sync.dma_start(sb[name][:], din[name].ap())
        for name, shape in [('imT', (81, NPAIR)), ('xP', (81, 3 * B))]:
            sb[name] = consts.tile(list(shape), DT, tag=name, name=f"c_{name}")

        cst_negthr = consts.tile([128, 1], DT, tag="cst_negthr")
        nc.vector.memset(cst_negthr[:], -THR)
        cst_eps = consts.tile([128, 1], DT, tag="cst_eps")
        nc.vector.memset(cst_eps[:], 1e-3)

        state = ctx.enter_context(tc.tile_pool(name="state", bufs=1))
        A = state.tile([128, SCH * NPAIR], DT, tag="A")      # y_tmp / y_new
        Bt = state.tile([128, SCH * NPAIR], DT, tag="B")     # y_last / y_mom
        nc.vector.memset(A[:], 0.0)
        nc.vector.memset(Bt[:], 0.0)

        scr = ctx.enter_context(tc.tile_pool(name="scr", bufs=2))
        sqp = ctx.enter_context(tc.tile_pool(name="sqp", bufs=1))
        epi = ctx.enter_context(tc.tile_pool(name="epi", bufs=1))
        xi = epi.tile([NPAIR, 73 * 73], DT, tag="xi")
        dram = ctx.enter_context(tc.tile_pool(name="dram", bufs=1, space="DRAM"))
        cc_in = dram.tile([3, 2], DT)
        cc_out = dram.tile([3, 2], DT)
        ag_in = dram.tile([NSH * 2, 81], DT)
        ag_out = dram.tile([NCORES * NSH * 2, 81], DT)

        # full x lands in SBUF [64,243] and is PE-transposed into xP at the
        # top of the epilogue; imT ships per-core (FISTA needs it at t=0).
        imT_src = (din['xc'].ap()[B:B + NSH, :]
                   .rearrange("a b -> (a b)")
                   .rearrange("(p q) -> p q", q=NPAIR))
        nc.sync.dma_start(sb['imT'][:], imT_src)
        xg_sb = epi.tile([B, 243], DT, tag="xg_sb")
        nc.sync.dma_start(xg_sb[:], din['xc'].ap()[0:B, :])

        # ---------------- FISTA ----------------
        with tc.tile_pool(name="ps_proj", bufs=2, space="PSUM") as ps_proj, \
             tc.tile_pool(name="ps_projT", bufs=2, space="PSUM") as ps_projT, \
             tc.tile_pool(name="ps_re", bufs=2, space="PSUM") as ps_re, \
             tc.tile_pool(name="ps_tr", bufs=2, space="PSUM") as ps_tr:
            for t in range(iters):
                if t == 0:
                    dT = sb['imT']
                else:
                    pj = ps_proj.tile([NPAIR, 81], DT, tag="pj")
                    for ci in range(SCH):
                        nc.tensor.matmul(
                            pj[:], mc(A[:, ci * NPAIR:(ci + 1) * NPAIR]),
                            mc(sb['mat_sb'][:, ci * 81:(ci + 1) * 81]),
                            start=(ci == 0), stop=(ci == SCH - 1))
                    pjs = scr.tile([NPAIR, 81], DT, tag="pjs")
                    nc.scalar.copy(pjs[:], pj[:])
                    pjT = ps_projT.tile([81, NPAIR], DT, tag="pjT")
                    nc.tensor.transpose(pjT[:], pjs[:], sb['ident'][:NPAIR, :NPAIR])
                    dT = scr.tile([81, NPAIR], DT, tag="dT")
                    nc.vector.tensor_tensor(dT[:], sb['imT'][:], pjT[:], OP.subtract)

                coef = float(coefs[t]) if coefs else 0.0
                last = (t == iters - 1)
                for g, (c0, cn) in enumerate(GRP):
                    re = ps_re.tile([128, 21 * NPAIR], DT, tag="re")
                    for j in range(cn):
                        ci = c0 + j
                        nc.tensor.matmul(
                            re[:, j * NPAIR:(j + 1) * NPAIR],
                            mc(sb['matT_sb'][:, ci * 128:(ci + 1) * 128]),
                            mc(dT[:]), start=True, stop=True)
                    sl = slice(c0 * NPAIR, (c0 + cn) * NPAIR)
                    rview = re[:, :cn * NPAIR]
                    W = scr.tile([128, 21 * NPAIR], DT, tag="W")
                    Wv = W[:, :cn * NPAIR]
                    nc.vector.tensor_tensor(Wv, A[:, sl], rview, OP.add)
                    P1 = scr.tile([128, 21 * NPAIR], DT, tag="P1")
                    P1v = P1[:, :cn * NPAIR]
                    nc.scalar.activation(P1v, Wv, AT.Relu, bias=cst_negthr[:])
                    P2 = scr.tile([128, 21 * NPAIR], DT, tag="P2")
                    P2v = P2[:, :cn * NPAIR]
                    nc.vector.tensor_scalar(P2v, Wv, THR, 0.0, OP.add, OP.min)
                    nc.vector.tensor_tensor(A[:, sl], P1v, P2v, OP.add)
                    if not last:
                        # y_mom = (y_new - y_last)*coef + y_new (reference order)
                        T = scr.tile([128, 21 * NPAIR], DT, tag="T")
                        Tv = T[:, :cn * NPAIR]
                        nc.vector.tensor_tensor(Tv, A[:, sl], Bt[:, sl], OP.subtract)
                        nc.vector.scalar_tensor_tensor(
                            Bt[:, sl], Tv, coef, A[:, sl], OP.mult, OP.add)
                A, Bt = Bt, A
            yfin = Bt if iters > 0 else A  # after swap, y_new lives in old-A

            # transposes into padded xi layout
            xiv = xi[:].rearrange("p (a b) -> p a b", b=73)
            for ci in range(SCH):
                tr = ps_tr.tile([NPAIR, 128], DT, tag="tr")
                nc.tensor.transpose(tr[:], yfin[:, ci * NPAIR:(ci + 1) * NPAIR],
                                    sb['ident'][:])
                s0, s1 = ci * 128, min(ci * 128 + 128, 5184)
                s = s0
                while s < s1:
                    a = s // 72
                    e = min(s1, (a + 1) * 72)
                    nc.vector.tensor_copy(
                        xiv[:, a + 1, s - a * 72 + 1:e - a * 72 + 1],
                        tr[:, s - s0:e - s0])
                    s = e
            nc.vector.tensor_copy(xiv[:, 0, 1:], xiv[:, 2, 1:])   # reflect row
            nc.vector.tensor_copy(xiv[:, :, 0], xiv[:, :, 2])     # reflect col+corner

        # ---------------- epilogue ----------------
        with tc.tile_pool(name="ps_mm", bufs=2, space="PSUM") as ps_mm, \
             tc.tile_pool(name="ps_c5", bufs=2, space="PSUM") as ps_c5:

            # build xP [81,(c b)] from gathered x via 3 PE transposes
            xgv = xg_sb[:].rearrange("b (p c) -> b p c", c=3)
            for c3 in range(3):
                pt = ps_mm.tile([81, B], DT, tag="mm")
                nc.tensor.transpose(pt[:], xgv[:, :, c3], sb['ident'][:B, :B])
                nc.vector.tensor_copy(sb['xP'][:, c3 * B:(c3 + 1) * B], pt[:])

            def bn_stats(src_ap, P, Fn, gather, bcast, gb, Nn, sq_tag):
                """returns alpha/beta tile [P,2] given pre-bn tensor [P,Fn]."""
                red = epi.tile([P, 2], DT, tag=sq_tag + "_red")
                nc.vector.tensor_reduce(red[:, 0:1], src_ap, mybir.AxisListType.X, OP.add)
                sq = sqp.tile([P, Fn], DT, tag="sq")
                nc.scalar.activation(sq[:P, :Fn], src_ap, AT.Square)
                nc.vector.tensor_reduce(red[:, 1:2], sq[:P, :Fn], mybir.AxisListType.X, OP.add)
                if gather is not None:
                    Cn = gather.shape[1]
                    ps = ps_mm.tile([Cn, 2], DT, tag="mm")
                    nc.tensor.matmul(ps[:], gather[:], red[:], start=True, stop=True)
                    st = epi.tile([Cn, 2], DT, tag=sq_tag + "_st")
                    nc.vector.tensor_copy(st[:], ps[:])
                else:
                    Cn = P
                    st = red
                return st, Cn

            def bn_alphabeta(st, Cn, gb, Nn, tagp):
                m = epi.tile([Cn, 1], DT, tag=tagp + "_m")
                nc.vector.tensor_scalar(m[:], st[:, 0:1], 1.0 / Nn, None, OP.mult)
                msq = epi.tile([Cn, 1], DT, tag=tagp + "_msq")
                nc.scalar.activation(msq[:], m[:], AT.Square)
                ve = epi.tile([Cn, 1], DT, tag=tagp + "_ve")
                nc.vector.scalar_tensor_tensor(ve[:], st[:, 1:2], 1.0 / Nn, msq[:],
                                               OP.mult, OP.subtract)
                sp = epi.tile([Cn, 1], DT, tag=tagp + "_sp")
                nc.scalar.activation(sp[:], ve[:], AT.Sqrt, bias=cst_eps[:Cn])
                istd = epi.tile([Cn, 1], DT, tag=tagp + "_is")
                nc.vector.reciprocal(istd[:], sp[:])
                ab = epi.tile([Cn, 2], DT, tag=tagp + "_ab")
                nc.vector.tensor_tensor(ab[:, 0:1], gb[:, 0:1], istd[:], OP.mult)
                am = epi.tile([Cn, 1], DT, tag=tagp + "_am")
                nc.vector.tensor_tensor(am[:], ab[:, 0:1], m[:], OP.mult)
                nc.vector.tensor_tensor(ab[:, 1:2], gb[:, 1:2], am[:], OP.subtract)
                return ab

            def bcast_ab(ab, bcast, P, tagp):
                ps = ps_mm.tile([P, 2], DT, tag="mm")
                nc.tensor.matmul(ps[:], bcast[:], ab[:], start=True, stop=True)
                abP = epi.tile([P, 2], DT, tag=tagp + "_abP")
                nc.vector.tensor_copy(abP[:], ps[:])
                return abP

            # ---- bn_x with AllReduce ----
            st3, _ = bn_stats(xi[:], NPAIR, 73 * 73, sb['C3sel'], None, None, None, "bx")
            nc.sync.dma_start(cc_in[:], st3[:])
            nc.gpsimd.collective_compute(
                "AllReduce", OP.add,
                replica_groups=[list(range(world))],
                ins=[cc_in.opt()], outs=[cc_out.opt()])
            g3 = epi.tile([3, 2], DT, tag="g3")
            nc.sync.dma_start(g3[:], cc_out[:])
            ab3 = bn_alphabeta(g3, 3, sb['bn_x_gb'], float(B * 73 * 73), "bx")
            ab24 = bcast_ab(ab3, sb['C3selT'], NPAIR, "bx")
            nc.vector.tensor_scalar(xi[:], xi[:], ab24[:, 0:1], ab24[:, 1:2],
                                    OP.mult, OP.add)

            # ---- conv5 + pools ----
            c5pad = epi.tile([NSH * 8, 72 * 72], DT, tag="c5pad")
            nc.gpsimd.memset(c5pad[:], -1e30)
            c5v = c5pad[:].rearrange("p (a b) -> p a b", b=72)
            ycs = [(i * 7, 7) for i in range(9)] + [(63, 6)]
            for yc, (y0, rows) in enumerate(ycs):
                ps = ps_c5.tile([NSH * 8, 7 * 69], DT, tag="c5")
                psv = ps[:, :rows * 69]
                for ti in range(25):
                    dy, dx = ti // 5, ti % 5
                    rhs = xiv[:, y0 + dy:y0 + dy + rows, dx:dx + 69]
                    nc.tensor.matmul(psv, mc(sb['w5taps'][:, ti * 64:(ti + 1) * 64]),
                                     mc(rhs), start=(ti == 0), stop=(ti == 24))
                dst = c5v[:, 1 + y0:1 + y0 + rows, 1:70]
                src = ps[:].rearrange("p (a b) -> p a b", b=69)[:, :rows, :]
                if yc % 2 == 0:
                    nc.vector.tensor_scalar(dst, src, sb['b5_bc'][:], None, OP.add)
                else:
                    nc.scalar.activation(dst, src, AT.Identity, bias=sb['b5_bc'][:])
            p4 = epi.tile([NSH * 8, 324], DT, tag="p4")
            pv = c5pad[:].rearrange("p (y a x b) -> p y x a b", y=18, a=4, x=18, b=4)
            nc.vector.tensor_reduce(p4[:], pv, mybir.AxisListType.XY, OP.max)
            psx = ps_mm.tile([NSH * 2, 324], DT, tag="mm")
            nc.tensor.matmul(psx[:], sb['wx2e'][:], p4[:], start=True, stop=True)
            xp2 = epi.tile([NSH * 2, 324], DT, tag="xp2")
            nc.scalar.activation(xp2[:], psx[:], AT.Relu, bias=sb['bx2_bc'][:])
            xo = epi.tile([NSH * 2, 81], DT, tag="xo")
            x2v = xp2[:].rearrange("p (y a x b) -> p y x a b", y=9, a=2, x=9, b=2)
            nc.vector.tensor_reduce(xo[:], x2v, mybir.AxisListType.XY, OP.max)
            nc.sync.dma_start(ag_in[:], xo[:])
            nc.gpsimd.collective_compute(
                "AllGather", OP.bypass,
                replica_groups=[list(range(world))],
                ins=[ag_in.opt()], outs=[ag_out.opt()])
            nc.sync.dma_start(out_all.ap()[0:128, 0:81], ag_out[:])

            # ---- w path ----
            def wsum3(cols, btag):
                t0 = epi.tile([81, B], DT, tag=btag + "_t0")
                nc.vector.tensor_scalar(t0[:], sb['xP'][:, 0:B],
                                        sb['smallw'][:, cols + 0:cols + 1], None, OP.mult)
                t1 = epi.tile([81, B], DT, tag=btag + "_t1")
                nc.vector.tensor_scalar(t1[:], sb['xP'][:, B:2 * B],
                                        sb['smallw'][:, cols + 1:cols + 2], None, OP.mult)
                nc.vector.tensor_tensor(t0[:], t0[:], t1[:], OP.add)
                nc.vector.tensor_scalar(t1[:], sb['xP'][:, 2 * B:3 * B],
                                        sb['smallw'][:, cols + 2:cols + 3], None, OP.mult)
                nc.vector.tensor_tensor(t0[:], t0[:], t1[:], OP.add)
                out = epi.tile([81, B], DT, tag=btag + "_o")
                nc.scalar.activation(out[:], t0[:], AT.Relu,
                                     bias=sb['smallw'][:, cols + 3:cols + 4])
                return out
            wi = wsum3(0, "wp")
            nc.sync.dma_start(out_all.ap()[0:81, 81:145], wi[:])

            # ---- y path ----
            y1 = wsum3(4, "yp")
            psy = ps_mm.tile([81, B], DT, tag="mm")
            nc.tensor.matmul(psy[:], sb['K7'][:], y1[:], start=True, stop=True)
            y7 = epi.tile([81, B], DT, tag="y7")
            nc.scalar.activation(y7[:], psy[:], AT.Identity, bias=sb['smallw'][:, 8:9])
            sty, _ = bn_stats(y7[:], 81, B, sb['ones81'], None, None, None, "by")
            aby = bn_alphabeta(sty, 1, sb['bn_y_gb'], float(81 * B), "by")
            aby81 = bcast_ab(aby, sb['onesT81'], 81, "by")
            yo = epi.tile([81, B], DT, tag="yo")
            nc.vector.tensor_scalar(yo[:], y7[:], aby81[:, 0:1], aby81[:, 1:2],
                                    OP.mult, OP.add)
            nc.sync.dma_start(out_all.ap()[0:81, 145:209], yo[:])

            # ---- z path ----
            psz1 = ps_mm.tile([108, B], DT, tag="mm")
            for c3 in range(3):
                nc.tensor.matmul(psz1[:], sb['W1big'][:, c3 * 108:(c3 + 1) * 108],
                                 sb['xP'][:, c3 * B:(c3 + 1) * B],
                                 start=(c3 == 0), stop=(c3 == 2))
            st1, _ = bn_stats(psz1[:], 108, B, sb['SZ1'], None, None, None, "b1")
            ab1 = bn_alphabeta(st1, 12, sb['bnd1_gb'], 576.0, "b1")
            ab1b = bcast_ab(ab1, sb['SZ1T'], 108, "b1")
            z1f = epi.tile([108, B], DT, tag="z1f")

            def leaky(dst, src_ap, ab, P, Fn, tagp):
                v = epi.tile([P, Fn], DT, tag=tagp + "_v")
                nc.vector.tensor_scalar(v[:], src_ap, ab[:, 0:1], ab[:, 1:2],
                                        OP.mult, OP.add)
                a = epi.tile([P, Fn], DT, tag=tagp + "_a")
                nc.scalar.activation(a[:], v[:], AT.Relu)
                b = epi.tile([P, Fn], DT, tag=tagp + "_b")
                nc.scalar.activation(b[:], v[:], AT.Relu, scale=-0.2)
                nc.vector.tensor_tensor(dst, a[:], b[:], OP.subtract)

            leaky(z1f[:], psz1[:], ab1b, 108, B, "l1")
            psz2 = ps_mm.tile([24, B], DT, tag="mm")
            nc.tensor.matmul(psz2[:], sb['wd2r'][:], z1f[:], start=True, stop=True)
            st2, _ = bn_stats(psz2[:], 24, B, None, None, None, None, "b2")
            ab2 = bn_alphabeta(st2, 24, sb['bnd2_gb'], float(B), "b2")
            z2f = epi.tile([24, B], DT, tag="z2f")
            leaky(z2f[:], psz2[:], ab2, 24, B, "l2")
            psu = ps_mm.tile([108, B], DT, tag="mm")
            nc.tensor.matmul(psu[:], sb['wu1r'][:], z2f[:], start=True, stop=True)
            zu = epi.tile([108, B], DT, tag="zu")
            nc.vector.tensor_copy(zu[:], psu[:])
            stu, _ = bn_stats(zu[:], 108, B, sb['SU'], None, None, None, "bu")
            abu = bn_alphabeta(stu, 12, sb['bnu1_gb'], float(9 * B), "bu")
            abu108 = bcast_ab(abu, sb['SUT'], 108, "bu")
            zuf = epi.tile([108, B], DT, tag="zuf")
            nc.scalar.activation(zuf[:], zu[:], AT.Relu,
                                 bias=abu108[:, 1:2], scale=abu108[:, 0:1])
            psf = ps_mm.tile([81, B], DT, tag="mm")
            nc.tensor.matmul(psf[:], sb['WU2up'][:], zuf[:], start=True, stop=False)
            nc.tensor.matmul(psf[:], sb['WU2z1'][:], z1f[:], start=False, stop=True)
            zo = epi.tile([81, B], DT, tag="zo")
            nc.scalar.activation(zo[:], psf[:], AT.Relu)
            nc.sync.dma_start(out_all.ap()[0:81, 209:273], zo[:])

    nc.compile()
    return nc


# --------------------------------------------------- persistent dispatch
def _make_dispatch(nc, static_np):
    """Build a persistent jitted shard_map runner + device-resident statics."""
    import jax
    import jax.numpy as jnp
    from jax.sharding import Mesh, PartitionSpec, NamedSharding
    from jax.experimental.shard_map import shard_map
    from concourse.bass2jax import (_bass_exec_p, install_neuronx_cc_hook,
                                    partition_id_tensor)

    install_neuronx_cc_hook()
    assert nc.dbg_addr is None
    partition_name = (nc.partition_id_tensor.name
                      if nc.partition_id_tensor else None)
    in_names, out_names, out_avals = [], [], []
    for alloc in nc.m.functions[0].allocations:
        if not isinstance(alloc, mybir.MemoryLocationSet):
            continue
        name = alloc.memorylocations[0].name
        if alloc.kind == "ExternalInput":
            if name != partition_name:
                in_names.append(name)
        elif alloc.kind == "ExternalOutput":
            out_names.append(name)
            out_avals.append(jax.core.ShapedArray(
                tuple(alloc.tensor_shape), mybir.dt.np(alloc.dtype)))
    n_params, n_outs = len(in_names), len(out_names)
    all_names = tuple(in_names + out_names +
                      ([partition_name] if partition_name else []))

    def _body(*args):
        operands = list(args)
        if partition_name is not None:
            operands.append(partition_id_tensor())
        return tuple(_bass_exec_p.bind(
            *operands, out_avals=tuple(out_avals), in_names=all_names,
            out_names=tuple(out_names), lowering_input_output_aliases=(),
            sim_require_finite=True, sim_require_nnan=True, nc=nc))

    devices = jax.devices()[:NCORES]
    mesh = Mesh(np.asarray(devices), ("core",))
    in_specs = (PartitionSpec("core"),) * (n_params + n_outs)
    out_specs = (PartitionSpec("core"),) * n_outs
    fn = jax.jit(shard_map(_body, mesh=mesh, in_specs=in_specs,
                           out_specs=out_specs, check_rep=False),
                 keep_unused=True)
    shard = NamedSharding(mesh, PartitionSpec("core"))

    # outputs are fully written by the kernel, so the zero "output operand"
    # buffers can persist across calls without donation
    def zeros_body():
        return tuple(jnp.zeros((NCORES * a.shape[0], *a.shape[1:]), a.dtype)
                     for a in out_avals)
    zeros_const = jax.jit(zeros_body, out_shardings=(shard,) * n_outs)()

    static_dev = {}
    for name, arr in static_np.items():
        g = np.ascontiguousarray(
            np.broadcast_to(arr, (NCORES, *arr.shape))
            .reshape(NCORES * arr.shape[0], *arr.shape[1:]))
        static_dev[name] = jax.device_put(g, shard)
    return dict(fn=fn, zeros_const=zeros_const, in_names=in_names,
                out_names=out_names, static_dev=static_dev, shard=shard)


# ----------------------------------------------------------------- kernel()
def _weights_key(inputs):
    return {k: np.asarray(v, F32) for k, v in inputs.items() if k != 'x'}


def _ensure_cache(inputs):
    w = _weights_key(inputs)
    if 'disp' in _CACHE and all(
            np.array_equal(w[k], _CACHE['weights'].get(k)) for k in w):
        return
    C = _host_shared(inputs)
    if 'nc' not in _CACHE:
        _CACHE['nc'] = _build(ITERS, C['coefs'])
    static_np = {name: C[name] for name, _ in SHARED_IN}
    _CACHE['disp'] = _make_dispatch(_CACHE['nc'], static_np)
    _CACHE['weights'] = w


def _run_once(x):
    d = _CACHE['disp']
    xc = np.empty((NCORES, B + NSH, 243), F32)
    xc[:, :B] = x.reshape(B, 243)
    xc[:, B:] = (x.reshape(NCORES, NSH, 81, 3).transpose(0, 2, 1, 3)
                 .reshape(NCORES, NSH, 243))
    percall = {'xc': xc.reshape(NCORES * (B + NSH), 243)}
    args = [percall[n] if n in percall else d['static_dev'][n]
            for n in d['in_names']]
    outs = d['fn'](*args, *d['zeros_const'])
    shard0 = next(s for s in outs[0].addressable_shards
                  if s.index[0].start in (0, None))
    return np.asarray(shard0.data)  # [128, 273], one round trip


def kernel(**inputs):
    x = np.asarray(inputs['x'], F32)
    _ensure_cache(inputs)
    try:
        buf = _run_once(x)
    except Exception:  # transient device/tunnel hiccup: retry once
        buf = _run_once(x)

    out = np.zeros((B, 9, 9, 5), F32)
    out[:, :, :, 1:3] = (buf[0:128, 0:81].reshape(NCORES, NSH, 2, 9, 9)
                         .transpose(0, 1, 3, 4, 2).reshape(B, 9, 9, 2))
    out[:, :, :, 0] = buf[0:81, 81:145].T.reshape(B, 9, 9)
    out[:, :, :, 3] = buf[0:81, 145:209].T.reshape(B, 9, 9)
    out[:, :, :, 4] = buf[0:81, 209:273].T.reshape(B, 9, 9)
    return out

